# revision 1
# baseline (speedup 1.0000x reference)
"""Trainium2 Bass kernel for a dense transformer block (B=2, T=2048, D=2048,
N=16 q heads, K=8 kv heads, H=128, F=8192, causal attention, RoPE, RMSNorm,
GeGLU FFN), sharded over 8 NeuronCores.

Sharding (Megatron-style TP + sequence-split FFN):
  - Each core owns 2 q heads + 1 kv head (column-split QKV with the pre-attn
    RMSNorm gain folded into the weights host-side).
  - All activations are kept feature-major (transposed, [feat, tok]) so that
    attention needs no P-matrix transposes: S^T = K @ Q^T comes straight from
    feature-major q/k, softmax denominators are ones-vector matmuls on the PE
    (no max subtraction needed: logits are O(5)), and P^T feeds the PV matmul
    directly.  Only V needs 32 small PE transposes.
  - attn_vec is row-split -> partial [D, 4096] per core, written chunk-blocked
    and ReduceScatter'd so core c ends with the full attention output for its
    512-token slice.
  - Each core then does residual + RMSNorm + the full-F FFN for its own
    512-token slice (same FLOPs as tensor-parallel FFN, no second collective),
    and returns out^T [2048, 512]; the host concatenates.
"""
import numpy as np

import concourse.bass as bass
import concourse.bacc as bacc
import concourse.tile as tile
from concourse import mybir
from concourse.bass_utils import run_bass_kernel_spmd
from concourse.masks import make_identity, make_upper_triangular

F32 = mybir.dt.float32
BF16 = mybir.dt.bfloat16
NP_BF16 = mybir.dt.np(BF16)

B, T, D = 2, 2048, 2048
N, K, H = 16, 8, 128
F = 8192
NCORES = 8
BT = B * T                  # 4096 flattened tokens (tok = b*T + t)
TOKC = BT // NCORES         # 512 tokens per chunk / per-core slice
NCHUNK = NCORES             # 8 token chunks
DT_TILES = D // 128         # 16
F_TILES = F // 128          # 64
GH = N // K                 # 2 q heads per kv head (= per core)
QB = T // TOKC              # 4 query chunks per batch
KVB = T // 128              # 16 kv blocks per batch
EPS = 1e-6


def build_program(n_cores=NCORES, sim=False, reps=1, no_rs=False):
    nc = bacc.Bacc("TRN2", target_bir_lowering=False, debug=False,
                   num_devices=n_cores)

    # ---- I/O ----
    xT_bf = nc.dram_tensor("xT_bf", [D, BT], BF16, kind="ExternalInput")
    xTs_f32 = nc.dram_tensor("xTs_f32", [D, TOKC], F32, kind="ExternalInput")
    wqkv = nc.dram_tensor("wqkv", [D, 4 * H], BF16, kind="ExternalInput")
    wav = nc.dram_tensor("wav", [N * H, D], BF16, kind="ExternalInput")
    wg0 = nc.dram_tensor("wg0", [D, F], BF16, kind="ExternalInput")
    wg1 = nc.dram_tensor("wg1", [D, F], BF16, kind="ExternalInput")
    wlin = nc.dram_tensor("wlin", [F, D], BF16, kind="ExternalInput")
    costab = nc.dram_tensor("costab", [64, BT], F32, kind="ExternalInput")
    sintab = nc.dram_tensor("sintab", [64, BT], F32, kind="ExternalInput")
    outT = nc.dram_tensor("outT", [D, TOKC], F32, kind="ExternalOutput")

    with tile.TileContext(nc) as tc:
        for _ in range(reps):
            _build(tc, (1 if no_rs else n_cores), sim, xT_bf, xTs_f32, wqkv,
                   wav, wg0, wg1, wlin, costab, sintab, outT)
    nc.compile()
    return nc


def _build(tc, n_cores, sim, xT_bf, xTs_f32, wqkv, wav, wg0, wg1, wlin,
           costab, sintab, outT):
    nc = tc.nc
    AF = mybir.ActivationFunctionType

    with tc.tile_pool(name="const", bufs=1) as const, \
         tc.tile_pool(name="dram", bufs=1, space="DRAM") as dram:
        ones128 = const.tile([128, 1], BF16, tag="ones128", name="ones128")
        nc.vector.memset(ones128[:], 1.0)
        ones_row = const.tile([1, 128], F32, tag="ones_row", name="ones_row")
        nc.vector.memset(ones_row[:], 1.0)
        ident = const.tile([128, 128], BF16, tag="ident", name="ident")
        make_identity(nc, ident[:])
        # keep-mask for diagonal attention blocks on S^T [kv, q]:
        # U[p, f] = 1 if p <= f else 0
        umask = const.tile([128, 128], BF16, tag="umask", name="umask")
        make_upper_triangular(nc, umask[:], val=1.0, diag=True)
        eps1 = const.tile([1, 1], F32, tag="eps1", name="eps1")
        nc.vector.memset(eps1[:], EPS)

        # DRAM buffers: collective in/out + act bounce
        cc_in = dram.tile([NCHUNK, 2 * H, TOKC], BF16,
                          tag="cc_in", name="cc_in")
        cc_out = dram.tile([NCHUNK, 2 * H, TOKC], BF16,
                           tag="cc_out", name="cc_out")
        res_dram = dram.tile([DT_TILES, 128, TOKC], F32,
                             tag="res_dram", name="res_dram")


        with tc.tile_pool(name="persAct", bufs=1) as pers:
            qT = [pers.tile([128, BT], BF16, tag=f"qT{h}", name=f"qT{h}")
                  for h in range(GH)]
            kT = pers.tile([128, BT], BF16, tag="kT", name="kT")
            vtok = [pers.tile([128, H], BF16, tag=f"vtok{g}", name=f"vtok{g}")
                    for g in range(2 * KVB)]
            encT = [pers.tile([128, BT], BF16, tag=f"encT{h}", name=f"encT{h}")
                    for h in range(GH)]

            _phase_proj(tc, nc, AF, xT_bf, wqkv, costab, sintab,
                        ones128, ones_row, ident, eps1, qT, kT, vtok)
            _phase_attn(tc, nc, AF, ones128, ones_row, umask, qT, kT, vtok,
                        encT)
            _phase_av_rs(tc, nc, n_cores, encT, cc_in, cc_out)

        _phase_ffn(tc, nc, AF, sim, xTs_f32, wav, wg0, wg1, wlin, cc_out,
                   res_dram, ones128, ones_row, eps1, outT)


def _phase_proj(tc, nc, AF, xT_bf, wqkv, costab, sintab,
                ones128, ones_row, ident, eps1, qT, kT, vtok):
    """RMSNorm stats + QKV projection + RoPE, feature-major."""
    with tc.tile_pool(name="ptab", bufs=1) as ptab, \
         tc.tile_pool(name="pAB", bufs=2) as pAB, \
         tc.tile_pool(name="pAB3", bufs=3) as pAB3, \
         tc.tile_pool(name="psAB", bufs=1, space="PSUM") as psAB:
        # resident qkv weights: 16 tiles [128, 512]
        wqkv_t = []
        for kt in range(DT_TILES):
            wt = ptab.tile([128, 4 * H], BF16, tag=f"wqkv{kt}",
                           name=f"wqkv{kt}")
            nc.sync.dma_start(wt[:], wqkv[kt * 128:(kt + 1) * 128, :])
            wqkv_t.append(wt)

        for c in range(NCHUNK):
            cols = slice(c * TOKC, (c + 1) * TOKC)
            xc = pAB.tile([128, DT_TILES, TOKC], BF16, tag="xc", name="xc")
            nc.sync.dma_start(
                xc[:], xT_bf[:, cols].rearrange("(kt p) n -> p kt n", p=128))

            # sum over D of x^2 via ones-matmul (cross-partition reduce)
            ssq = psAB.tile([1, TOKC], F32, tag="ssq", name="ssq")
            for kt in range(DT_TILES):
                sq = pAB3.tile([128, TOKC], BF16, tag="sq", name="sq")
                nc.vector.tensor_mul(sq[:], xc[:, kt, :], xc[:, kt, :])
                nc.tensor.matmul(ssq[:], ones128[:], sq[:],
                                 start=(kt == 0), stop=(kt == DT_TILES - 1))
            # r = 1/sqrt(mean + eps)
            sd = pAB.tile([1, TOKC], F32, tag="sd", name="sd")
            nc.scalar.activation(sd[:], ssq[:], AF.Sqrt,
                                 bias=eps1[:], scale=1.0 / D)
            rr = pAB.tile([1, TOKC], F32, tag="rr", name="rr")
            nc.vector.reciprocal(rr[:], sd[:])
            # broadcast r across partitions via K=1 matmul
            rb_ps = psAB.tile([128, TOKC], F32, tag="rb_ps", name="rb_ps")
            nc.tensor.matmul(rb_ps[:], ones_row[:], rr[:],
                             start=True, stop=True)
            rb = pAB.tile([128, TOKC], F32, tag="rb", name="rb")
            nc.vector.tensor_copy(rb[:], rb_ps[:])
            # r-folded rope tables for this chunk
            cos_c = pAB.tile([64, TOKC], F32, tag="cos_c", name="cos_c")
            nc.sync.dma_start(cos_c[:], costab[:, cols])
            sin_c = pAB.tile([64, TOKC], F32, tag="sin_c", name="sin_c")
            nc.sync.dma_start(sin_c[:], sintab[:, cols])
            cosr = pAB.tile([64, TOKC], F32, tag="cosr", name="cosr")
            nc.vector.tensor_mul(cosr[:], cos_c[:], rb[0:64, :])
            sinr = pAB.tile([64, TOKC], F32, tag="sinr", name="sinr")
            nc.vector.tensor_mul(sinr[:], sin_c[:], rb[0:64, :])

            # qkv projection: 4 psum tiles [128, TOKC]
            proj = [psAB.tile([128, TOKC], F32, tag=f"proj{ft}",
                              name=f"proj{ft}") for ft in range(4)]
            for kt in range(DT_TILES):
                for ft in range(4):
                    nc.tensor.matmul(
                        proj[ft][:],
                        wqkv_t[kt][:, ft * 128:(ft + 1) * 128],
                        xc[:, kt, :],
                        start=(kt == 0), stop=(kt == DT_TILES - 1))

            # rope for q heads and k (r folded into the tables)
            for ft in range(3):
                dst = qT[ft] if ft < GH else kT
                ps = proj[ft]
                t1 = pAB.tile([64, TOKC], F32, tag="t1", name="t1")
                t2 = pAB.tile([64, TOKC], F32, tag="t2", name="t2")
                nc.vector.tensor_mul(t1[:], ps[0:64, :], cosr[:])
                nc.vector.tensor_mul(t2[:], ps[64:128, :], sinr[:])
                nc.vector.tensor_sub(dst[0:64, cols], t1[:], t2[:])
                t3 = pAB.tile([64, TOKC], F32, tag="t3", name="t3")
                t4 = pAB.tile([64, TOKC], F32, tag="t4", name="t4")
                nc.vector.tensor_mul(t3[:], ps[64:128, :], cosr[:])
                nc.vector.tensor_mul(t4[:], ps[0:64, :], sinr[:])
                nc.vector.tensor_add(dst[64:128, cols], t3[:], t4[:])

            # v: plain r scaling, then transpose to token-major
            vsb = pAB.tile([128, TOKC], BF16, tag="vsb", name="vsb")
            nc.vector.tensor_mul(vsb[:], proj[3][:], rb[:])
            for j in range(TOKC // 128):
                vt_ps = psAB.tile([128, 128], BF16, tag="vt_ps", name="vt_ps")
                nc.tensor.transpose(vt_ps[:], vsb[:, j * 128:(j + 1) * 128],
                                    ident[:])
                nc.vector.tensor_copy(vtok[c * 4 + j][:], vt_ps[:])


def _phase_attn(tc, nc, AF, ones128, ones_row, umask, qT, kT, vtok, encT):
    """Causal attention in the transposed (S^T) formulation."""
    with tc.tile_pool(name="pC", bufs=3) as pC, \
         tc.tile_pool(name="pC2", bufs=2) as pC2, \
         tc.tile_pool(name="psC", bufs=2, space="PSUM") as psC, \
         tc.tile_pool(name="psC1", bufs=1, space="PSUM") as psC1:
        for b in range(B):
            for h in range(GH):
                for c in range(QB):
                    cq = b * QB + c
                    qcols = slice(cq * TOKC, (cq + 1) * TOKC)
                    o_ps = psC.tile([128, TOKC], F32, tag="o_ps", name="o_ps")
                    den_ps = psC1.tile([1, TOKC], F32, tag="den_ps",
                                       name="den_ps")
                    nkv = 4 * c + 4   # kv blocks 0 .. 4c+3
                    for j in range(nkv):
                        g = b * KVB + j
                        s_ps = psC.tile([128, TOKC], F32, tag="s_ps",
                                        name="s_ps")
                        nc.tensor.matmul(
                            s_ps[:], kT[:, g * 128:(g + 1) * 128],
                            qT[h][:, qcols], start=True, stop=True)
                        p_sb = pC.tile([128, TOKC], BF16, tag="p_sb",
                                       name="p_sb")
                        nc.scalar.activation(p_sb[:], s_ps[:], AF.Exp)
                        d = j - 4 * c
                        if d >= 0:
                            # blocks left of the diagonal are fully masked
                            if d > 0:
                                nc.vector.memset(p_sb[:, 0:d * 128], 0.0)
                            nc.vector.tensor_mul(
                                p_sb[:, d * 128:(d + 1) * 128],
                                p_sb[:, d * 128:(d + 1) * 128], umask[:])
                        nc.tensor.matmul(den_ps[:], ones128[:], p_sb[:],
                                         start=(j == 0), stop=(j == nkv - 1))
                        nc.tensor.matmul(o_ps[:], vtok[g][:], p_sb[:],
                                         start=(j == 0), stop=(j == nkv - 1))
                    # normalize: enc = O / denom
                    rec = pC2.tile([1, TOKC], F32, tag="rec", name="rec")
                    nc.vector.reciprocal(rec[:], den_ps[:])
                    db_ps = psC1.tile([128, TOKC], F32, tag="db_ps",
                                      name="db_ps")
                    nc.tensor.matmul(db_ps[:], ones_row[:], rec[:],
                                     start=True, stop=True)
                    db = pC2.tile([128, TOKC], F32, tag="db", name="db")
                    nc.vector.tensor_copy(db[:], db_ps[:])
                    nc.vector.tensor_mul(encT[h][:, qcols], o_ps[:], db[:])


def _phase_av_rs(tc, nc, n_cores, encT, cc_in, cc_out):
    """AllToAll of per-head enc: core c keeps heads 2c,2c+1 for all tokens;
    after A2A it holds ALL 16 heads' enc for its own 512-token slice."""
    for cq in range(NCHUNK):
        qcols = slice(cq * TOKC, (cq + 1) * TOKC)
        for h in range(GH):
            nc.sync.dma_start(cc_in[cq, h * H:(h + 1) * H, :],
                              encT[h][:, qcols])
    if n_cores > 1:
        nc.gpsimd.collective_compute(
            "AllToAll",
            mybir.AluOpType.bypass,
            replica_groups=[list(range(n_cores))],
            ins=[cc_in.opt()],
            outs=[cc_out.opt()],
        )
    else:
        nc.sync.dma_start(cc_out[:], cc_in[:])


def _phase_ffn(tc, nc, AF, sim, xTs_f32, wav, wg0, wg1, wlin, cc_out,
               res_dram, ones128, ones_row, eps1, outT):
    """residual + RMSNorm + GeGLU FFN on this core's 512-token slice."""
    with tc.tile_pool(name="pE", bufs=2) as pE, \
         tc.tile_pool(name="pE1", bufs=1) as pE1, \
         tc.tile_pool(name="pE3", bufs=2) as pE3:
        hn = [pE1.tile([128, TOKC], BF16, tag=f"hn{dt}", name=f"hn{dt}")
              for dt in range(DT_TILES)]
        with tc.tile_pool(name="pAV", bufs=1) as pAV, \
             tc.tile_pool(name="psE", bufs=1, space="PSUM") as psE, \
             tc.tile_pool(name="psAV", bufs=2, space="PSUM") as psAV:
            # gather all heads' enc for my slice; attn_vec with full w_av
            wavf, encf = [], []
            for kt in range(DT_TILES):
                wt = pAV.tile([128, D], BF16, tag=f"wavf{kt}",
                              name=f"wavf{kt}")
                nc.sync.dma_start(wt[:], wav[kt * 128:(kt + 1) * 128, :])
                wavf.append(wt)
            for j in range(NCHUNK):
                for h in range(GH):
                    ef = pAV.tile([128, TOKC], BF16, tag=f"encf{2*j+h}",
                                  name=f"encf{2*j+h}")
                    nc.sync.dma_start(ef[:], cc_out[j, h * H:(h + 1) * H, :])
                    encf.append(ef)
            ssq2 = psE.tile([1, TOKC], F32, tag="ssq2", name="ssq2")
            for dt in range(DT_TILES):
                ao_ps = psAV.tile([128, TOKC], F32, tag="ao_ps", name="ao_ps")
                for kt in range(DT_TILES):
                    nc.tensor.matmul(ao_ps[:],
                                     wavf[kt][:, dt * 128:(dt + 1) * 128],
                                     encf[kt][:],
                                     start=(kt == 0),
                                     stop=(kt == DT_TILES - 1))
                xs = pE.tile([128, TOKC], F32, tag="xs", name="xs")
                nc.sync.dma_start(xs[:], xTs_f32[dt * 128:(dt + 1) * 128, :])
                rt = pE3.tile([128, TOKC], F32, tag="rt", name="rt")
                nc.vector.tensor_add(rt[:], ao_ps[:], xs[:])
                nc.sync.dma_start(res_dram[dt], rt[:])
                sq2 = pE3.tile([128, TOKC], BF16, tag="sq2", name="sq2")
                nc.vector.tensor_mul(sq2[:], rt[:], rt[:])
                nc.tensor.matmul(ssq2[:], ones128[:], sq2[:],
                                 start=(dt == 0), stop=(dt == DT_TILES - 1))
            sd2 = pE.tile([1, TOKC], F32, tag="sd2", name="sd2")
            nc.scalar.activation(sd2[:], ssq2[:], AF.Sqrt,
                                 bias=eps1[:], scale=1.0 / D)
            rr2 = pE.tile([1, TOKC], F32, tag="rr2", name="rr2")
            nc.vector.reciprocal(rr2[:], sd2[:])
            r2b_ps = psE.tile([128, TOKC], F32, tag="r2b_ps", name="r2b_ps")
            nc.tensor.matmul(r2b_ps[:], ones_row[:], rr2[:],
                             start=True, stop=True)
            r2b = pE1.tile([128, TOKC], F32, tag="r2b", name="r2b")
            nc.vector.tensor_copy(r2b[:], r2b_ps[:])
            for dt in range(DT_TILES):
                rl = pE.tile([128, TOKC], F32, tag="rl", name="rl")
                nc.sync.dma_start(rl[:], res_dram[dt])
                nc.vector.tensor_mul(hn[dt][:], rl[:], r2b[:])

        # gate path: act = gelu_tanh(hn @ wg0) * (hn @ wg1), act kept in SBUF
        pActs_cm = tc.tile_pool(name="pActs", bufs=1)
        pActs = pActs_cm.__enter__()
        act = [pActs.tile([128, TOKC], BF16, tag=f"act{f}", name=f"act{f}")
               for f in range(F_TILES)]
        with tc.tile_pool(name="pW", bufs=2) as pW, \
             tc.tile_pool(name="psW", bufs=2, space="PSUM") as psW:
            for fs in range(F // 512):
                w0s, w1s = [], []
                for kt in range(DT_TILES):
                    w0 = pW.tile([128, 512], BF16, tag=f"w0s{kt}",
                                 name=f"w0s{kt}")
                    nc.sync.dma_start(
                        w0[:], wg0[kt * 128:(kt + 1) * 128,
                                   fs * 512:(fs + 1) * 512])
                    w0s.append(w0)
                    w1 = pW.tile([128, 512], BF16, tag=f"w1s{kt}",
                                 name=f"w1s{kt}")
                    nc.sync.dma_start(
                        w1[:], wg1[kt * 128:(kt + 1) * 128,
                                   fs * 512:(fs + 1) * 512])
                    w1s.append(w1)
                for fi in range(4):
                    f = fs * 4 + fi
                    g_ps = psW.tile([128, TOKC], F32, tag="g_ps", name="g_ps")
                    u_ps = psW.tile([128, TOKC], F32, tag="u_ps", name="u_ps")
                    for kt in range(DT_TILES):
                        nc.tensor.matmul(g_ps[:],
                                         w0s[kt][:, fi * 128:(fi + 1) * 128],
                                         hn[kt][:],
                                         start=(kt == 0),
                                         stop=(kt == DT_TILES - 1))
                    for kt in range(DT_TILES):
                        nc.tensor.matmul(u_ps[:],
                                         w1s[kt][:, fi * 128:(fi + 1) * 128],
                                         hn[kt][:],
                                         start=(kt == 0),
                                         stop=(kt == DT_TILES - 1))
                    gg = pE3.tile([128, TOKC], BF16, tag="gg", name="gg")
                    if not sim:
                        nc.scalar.activation(gg[:], g_ps[:],
                                             AF.Gelu_apprx_tanh)
                    else:
                        # tanh-gelu composite (CoreSim has no Gelu LUT)
                        x2 = pE3.tile([128, TOKC], F32, tag="x2", name="x2")
                        nc.vector.tensor_mul(x2[:], g_ps[:], g_ps[:])
                        x3 = pE3.tile([128, TOKC], F32, tag="x3", name="x3")
                        nc.vector.tensor_mul(x3[:], x2[:], g_ps[:])
                        inner = pE3.tile([128, TOKC], F32, tag="inner",
                                         name="inner")
                        nc.vector.tensor_scalar(inner[:], x3[:], 0.044715,
                                                None, mybir.AluOpType.mult)
                        nc.vector.tensor_add(inner[:], inner[:], g_ps[:])
                        th = pE3.tile([128, TOKC], F32, tag="th", name="th")
                        nc.scalar.activation(th[:], inner[:], AF.Tanh,
                                             scale=0.7978845608028654)
                        nc.vector.tensor_scalar(th[:], th[:], 1.0, 0.5,
                                                mybir.AluOpType.add,
                                                mybir.AluOpType.mult)
                        nc.vector.tensor_mul(gg[:], th[:], g_ps[:])
                    nc.vector.tensor_mul(act[f][:], u_ps[:], gg[:])

        # linear: out^T[dt] = sum_f wlin[f, dt-cols].T @ act[f]  (+ residual)
        with tc.tile_pool(name="pL", bufs=8) as pL, \
             tc.tile_pool(name="psL", bufs=1, space="PSUM") as psL:
            for pas in range(2):       # dt 0-7, then 8-15
                o_ps = [psL.tile([128, TOKC], F32, tag=f"o_ps{i}",
                                 name=f"o_ps{i}") for i in range(8)]
                for f in range(F_TILES):
                    for half in range(2):
                        col0 = pas * 1024 + half * 512
                        wl = pL.tile([128, 512], BF16, tag=f"wls{half}",
                                     name=f"wls{half}")
                        nc.sync.dma_start(
                            wl[:], wlin[f * 128:(f + 1) * 128,
                                        col0:col0 + 512])
                        for i in range(4):
                            nc.tensor.matmul(
                                o_ps[half * 4 + i][:],
                                wl[:, i * 128:(i + 1) * 128], act[f][:],
                                start=(f == 0), stop=(f == F_TILES - 1))
                for i in range(8):
                    dt = pas * 8 + i
                    rl2 = pE3.tile([128, TOKC], F32, tag="rl2", name="rl2")
                    nc.sync.dma_start(rl2[:], res_dram[dt])
                    ob = pE3.tile([128, TOKC], F32, tag="ob", name="ob")
                    nc.vector.tensor_add(ob[:], o_ps[i][:], rl2[:])
                    nc.sync.dma_start(outT[dt * 128:(dt + 1) * 128, :], ob[:])
        pActs_cm.__exit__(None, None, None)


# ---------------------------------------------------------------------------
# Host side
# ---------------------------------------------------------------------------
def make_host_inputs(x, positions, w_q, w_kv, w_attn_vec, scale_pre_attn,
                     scale_pre_ffw, w_gating, w_linear):
    """Build the per-core input maps (all numpy)."""
    x = np.asarray(x, np.float32)
    positions = np.asarray(positions)
    w_q = np.asarray(w_q, np.float32)
    w_kv = np.asarray(w_kv, np.float32)
    w_attn_vec = np.asarray(w_attn_vec, np.float32)
    s1 = 1.0 + np.asarray(scale_pre_attn, np.float32)
    s2 = 1.0 + np.asarray(scale_pre_ffw, np.float32)
    w_gating = np.asarray(w_gating, np.float32)
    w_linear = np.asarray(w_linear, np.float32)

    xT = np.ascontiguousarray(x.reshape(BT, D).T)          # [D, BT] f32
    xT_bf = xT.astype(NP_BF16)

    pos = positions.reshape(BT).astype(np.float32)         # [BT]
    half = H // 2
    timescale = (10000.0 ** ((2.0 / H) * np.arange(half, dtype=np.float32)))
    rad = pos[None, :] / timescale[:, None]                # [64, BT]
    costab = np.cos(rad).astype(np.float32)
    sintab = np.sin(rad).astype(np.float32)

    wg0 = np.ascontiguousarray((w_gating[0] * s2[:, None]).astype(NP_BF16))
    wg1 = np.ascontiguousarray((w_gating[1] * s2[:, None]).astype(NP_BF16))
    wlin = np.ascontiguousarray(w_linear.astype(NP_BF16))
    wav_full = np.ascontiguousarray(
        w_attn_vec.reshape(N * H, D).astype(NP_BF16))

    in_maps = []
    for c in range(NCORES):
        hq0, hq1 = 2 * c, 2 * c + 1
        wq0 = w_q[hq0] * s1[:, None] * (H ** -0.5)
        wq1 = w_q[hq1] * s1[:, None] * (H ** -0.5)
        wk = w_kv[0, c] * s1[:, None]
        wv = w_kv[1, c] * s1[:, None]
        wqkv_c = np.concatenate([wq0, wq1, wk, wv], axis=1).astype(NP_BF16)
        wav_c = wav_full
        xTs = np.ascontiguousarray(xT[:, c * TOKC:(c + 1) * TOKC])
        in_maps.append({
            "xT_bf": xT_bf,
            "xTs_f32": xTs,
            "wqkv": np.ascontiguousarray(wqkv_c),
            "wav": wav_c,
            "wg0": wg0,
            "wg1": wg1,
            "wlin": wlin,
            "costab": costab,
            "sintab": sintab,
        })
    return in_maps


def assemble_output(results):
    """results: list of per-core {"outT": [D, TOKC] f32} -> [B, T, D] f32."""
    outT = np.concatenate([np.asarray(r["outT"]) for r in results], axis=1)
    return np.ascontiguousarray(outT.T.reshape(B, T, D)).astype(np.float32)


_CACHE = {}


def _get_program():
    if "nc" not in _CACHE:
        _CACHE["nc"] = build_program(NCORES)
    return _CACHE["nc"]


def kernel(x, positions, attn_mask, w_q, w_kv, w_attn_vec, scale_pre_attn,
           scale_pre_ffw, w_gating, w_linear):
    nc = _get_program()
    in_maps = make_host_inputs(x, positions, w_q, w_kv, w_attn_vec,
                               scale_pre_attn, scale_pre_ffw, w_gating,
                               w_linear)
    _CACHE["in_maps"] = in_maps
    res = run_bass_kernel_spmd(nc, in_maps, list(range(NCORES)))
    return assemble_output(res.results)

